# revision 14
# baseline (speedup 1.0000x reference)
"""GCN message passing (SpMM) on 8 Trainium2 NeuronCores.

out[r, :] = sum_{e: rows[e]==r} vals[e] * x[cols[e], :]  (N=100000, D=48,
E=1.6M, rows sorted).

Sharding: 1D row partitioning. Core k owns output rows [k*12500, (k+1)*12500)
and the contiguous edge range hitting those rows. No collectives.

Per-core algorithm (v2, dense triple-group direct-write):
  - Output rows are grouped into fixed "triples" of 96 consecutive rows;
    a chunk is 10 triples (960 rows); 14 chunks cover 12500 rows (padded).
  - x is padded to [100000, 64] f32 (256B rows) and split into 4 node-range
    buckets of 25000 rows so dma_gather's int16 indices can address each.
  - Per (chunk, bucket) the edges are sorted by (triple, col) and cut into
    128-edge gather tiles (no per-window padding; only the last tile of
    each (chunk, bucket) pads, and whole pad tiles equalize cores).
  - Each tile emits one matmul per triple its edges span (usually 1, ~1.3
    avg): out_psum[96 rows of triple, 48] += S^T @ G, where S [128, 96] is
    a masked selector (vals at (edge, row%96), zero for edges of other
    triples) built by DVE from per-matmul slot/val metadata via iota
    compare, and G is the gathered tile.  Spans are unioned across cores so
    all 8 cores share one program.
  - PSUM [96, 10*48] accumulates a whole chunk; ACT copies it to SBUF and a
    plain strided DMA writes 960 dense rows to HBM (no dma_scatter_add).
  - Gathers run on 4 SWDGE queues (one per bucket); metadata is preloaded
    to SBUF once.
"""

import numpy as np

import concourse.bass as bass
import concourse.bacc as bacc
import concourse.mybir as mybir
import concourse.tile as tile
from concourse.bass_utils import run_bass_kernel_spmd

# ---------------- problem constants (hardcoded per the task contract) -------
N_NODES = 100000
D = 48
N_CORES = 8
R_PER_CORE = N_NODES // N_CORES  # 12500

# ---------------- kernel hyperparameters -----------------------------------
NB = 4                 # node-range buckets (int16 gather indices)
B_NODES = N_NODES // NB
TRI = 96               # rows per triple (= PSUM partition group)
TPC = 10               # triples per chunk (10*48 f32 = 1920B <= 2KB PSUM bank)
RPCH = TRI * TPC       # rows per chunk = 960
N_CHUNKS = -(-R_PER_CORE // RPCH)          # 14
R_PAD = N_CHUNKS * RPCH                    # 13440
EL = 64                # padded x row, f32 elements (256B)
PAD_SLOT = 127.0       # slot id for pad edges (no iota(96) match)

_F32 = mybir.dt.float32
_I16 = mybir.dt.int16


def _wrap16(flat, reps=8):
    """[(n)] int16 -> [16*reps, n/16] in the 16-partition wrap, replicated."""
    n = flat.shape[0]
    w = flat.reshape(n // 16, 16).T  # [16, n/16]
    return np.tile(w, (reps, 1))


# ===========================================================================
# Host-side prep: pure index/layout transformation (no float math on data).
# ===========================================================================
def prep_inputs(adj_rows, adj_cols, adj_vals):
    """Shard + pack.  Returns (in_maps, plan) where plan drives build."""
    adj_rows = np.asarray(adj_rows).astype(np.int64)
    adj_cols = np.asarray(adj_cols).astype(np.int64)
    adj_vals = np.asarray(adj_vals).astype(np.float32)

    bounds = np.searchsorted(adj_rows, np.arange(N_CORES + 1) * R_PER_CORE)
    # per-core sorted edge structures
    cores = []
    for k in range(N_CORES):
        e0, e1 = bounds[k], bounds[k + 1]
        r = adj_rows[e0:e1] - k * R_PER_CORE
        c = adj_cols[e0:e1]
        v = adj_vals[e0:e1]
        b = c // B_NODES
        cl = (c - b * B_NODES).astype(np.int64)
        tri = r // TRI                    # global triple id (0..139)
        ch = tri // TPC                   # chunk id
        slot = (r % TRI).astype(np.float32)
        o = np.lexsort((cl, tri, b * N_CHUNKS + ch))
        cores.append((b[o], ch[o], tri[o], cl[o], slot[o], v[o]))

    # tiles per (chunk, bucket): max over cores
    T_cb = np.zeros((N_CHUNKS, NB), np.int64)
    cnt_kcb = np.zeros((N_CORES, N_CHUNKS, NB), np.int64)
    for k in range(N_CORES):
        b, ch = cores[k][0], cores[k][1]
        np.add.at(cnt_kcb[k], (ch, b), 1)
    T_cb = -(-cnt_kcb.max(axis=0) // 128)

    # per (c,b,t): union of spanned triples across cores -> matmul plan
    spans = {}
    for k in range(N_CORES):
        b, ch, tri = cores[k][0], cores[k][1], cores[k][2]
        # edge position within its (c,b) group
        start = {}
        pos = 0
        # groups appear in sorted order (b-major, then chunk)
        # compute group start offsets
        grp = b * N_CHUNKS + ch
        # since sorted by grp, find boundaries
        gb = np.searchsorted(grp, np.arange(NB * N_CHUNKS))
        ge = np.searchsorted(grp, np.arange(NB * N_CHUNKS), side="right")
        for bb in range(NB):
            for cc in range(N_CHUNKS):
                g0, g1 = gb[bb * N_CHUNKS + cc], ge[bb * N_CHUNKS + cc]
                tt = tri[g0:g1]
                for t in range((g1 - g0 + 127) // 128):
                    seg = tt[t * 128:(t + 1) * 128]
                    key = (cc, bb, t)
                    s = spans.setdefault(key, set())
                    s.update(np.unique(seg).tolist())

    # plan[c] = list of (b, t, j, m_local(b), start, stop)
    plan = []
    M_cb = np.zeros((N_CHUNKS, NB), np.int64)
    for c in range(N_CHUNKS):
        entries = []   # (b, t, j)
        for b in range(NB):
            m = 0
            for t in range(T_cb[c, b]):
                taus = sorted(spans.get((c, b, t), {c * TPC}))
                for tau in taus:
                    entries.append((b, t, tau - c * TPC, m))
                    m += 1
            M_cb[c, b] = m
        # a PSUM group's start..stop matmuls must be consecutive on PE:
        # order the chunk's matmuls by group j
        entries.sort(key=lambda e: (e[2], e[0], e[1]))
        # start/stop flags per j group
        first = {}
        last = {}
        for i, (b, t, j, m) in enumerate(entries):
            first.setdefault(j, i)
            last[j] = i
        # ensure every j group is written (init with a dummy if missing)
        missing = [j for j in range(TPC) if j not in first]
        plan.append({
            "entries": [(b, t, j, m, i == first[j], i == last[j])
                        for i, (b, t, j, m) in enumerate(entries)],
            "missing": missing,
        })

    # missing PSUM groups may only cover pad rows (sliced off by host)
    for c, p in enumerate(plan):
        for j in p["missing"]:
            assert c * RPCH + j * TRI >= R_PER_CORE, (c, j)

    # metadata arrays per core
    in_maps = []
    gi_w = (T_cb * 128 // 16)               # int16 cols per (c,b)
    gi_off = np.concatenate([[0], np.cumsum(gi_w.reshape(-1))])
    m_off = np.concatenate([[0], np.cumsum(M_cb.reshape(-1))])
    M_total = int(m_off[-1])
    GIW_total = int(gi_off[-1])

    iota = np.broadcast_to(np.arange(TRI, dtype=np.float32),
                           (128, TRI)).copy()
    for k in range(N_CORES):
        b, ch, tri, cl, slot, val = cores[k]
        grp = b * N_CHUNKS + ch
        gb = np.searchsorted(grp, np.arange(NB * N_CHUNKS))
        ge = np.searchsorted(grp, np.arange(NB * N_CHUNKS), side="right")
        gidx = np.zeros((128, GIW_total), np.int16)
        gslot = np.full((128, M_total), PAD_SLOT, np.float32)
        gval = np.zeros((128, M_total), np.float32)
        for c in range(N_CHUNKS):
            for bb in range(NB):
                g0, g1 = gb[bb * N_CHUNKS + c], ge[bb * N_CHUNKS + c]
                n = g1 - g0
                cap = int(T_cb[c, bb]) * 128
                idx = np.zeros(cap, np.int16)
                idx[:n] = cl[g0:g1]
                tr = np.full(cap, -1, np.int64)
                tr[:n] = tri[g0:g1]
                sl = np.full(cap, PAD_SLOT, np.float32)
                sl[:n] = slot[g0:g1]
                vv = np.zeros(cap, np.float32)
                vv[:n] = val[g0:g1]
                gidx[:, gi_off[c * NB + bb]:gi_off[c * NB + bb + 1]] = \
                    _wrap16(idx)
                # per-matmul slot/val columns
                mo = m_off[c * NB + bb]
                for (b2, t, j, m, st, sp) in plan[c]["entries"]:
                    if b2 != bb:
                        continue
                    tau = j + c * TPC
                    seg = np.arange(t * 128, (t + 1) * 128)
                    mine = tr[seg] == tau
                    gslot[:, mo + m] = np.where(mine, sl[seg], PAD_SLOT)
                    gval[:, mo + m] = np.where(mine, vv[seg], 0.0)
        in_maps.append({"gidx": gidx, "gslot": gslot, "gval": gval,
                        "iota": iota})

    meta = {"T_cb": T_cb, "M_cb": M_cb, "gi_off": gi_off, "m_off": m_off,
            "M_total": M_total, "GIW_total": GIW_total, "plan": plan}
    return in_maps, meta


def pad_x(x):
    x64 = np.zeros((N_NODES, EL), np.float32)
    x64[:, :D] = x
    return x64


# ===========================================================================
# Device program (shared across all 8 cores)
# ===========================================================================
def build_program(meta, repeat=1):
    T_cb = meta["T_cb"]
    M_cb = meta["M_cb"]
    gi_off = meta["gi_off"]
    m_off = meta["m_off"]
    plan = meta["plan"]

    nc = bacc.Bacc("TRN2", target_bir_lowering=False, debug=False,
                   num_devices=N_CORES, num_swdge_queues=4)
    x_d = nc.dram_tensor("x64", [N_NODES, EL], _F32, kind="ExternalInput")
    gidx_d = nc.dram_tensor("gidx", [128, meta["GIW_total"]], _I16,
                            kind="ExternalInput")
    gslot_d = nc.dram_tensor("gslot", [128, meta["M_total"]], _F32,
                             kind="ExternalInput")
    gval_d = nc.dram_tensor("gval", [128, meta["M_total"]], _F32,
                            kind="ExternalInput")
    iota_d = nc.dram_tensor("iota", [128, TRI], _F32, kind="ExternalInput")
    out_d = nc.dram_tensor("out", [R_PAD, D], _F32, kind="ExternalOutput")

    with tile.TileContext(nc) as tc:
        with (
            tc.tile_pool(name="meta", bufs=1) as mpool,
            tc.tile_pool(name="gbuf", bufs=2) as gbuf,
            tc.tile_pool(name="sbuf_s", bufs=1) as sbuf_s,
            tc.tile_pool(name="scp", bufs=2) as scp,
            tc.tile_pool(name="psum", bufs=2, space="PSUM") as psum,
        ):
          for _rep in range(repeat):
            iota_t = mpool.tile([128, TRI], _F32, tag="iota")
            nc.sync.dma_start(out=iota_t[:], in_=iota_d[:])
            gi_all = mpool.tile([128, meta["GIW_total"]], _I16, tag="giA")
            nc.sync.dma_start(out=gi_all[:], in_=gidx_d[:])
            gs_all = mpool.tile([128, meta["M_total"]], _F32, tag="gsA")
            nc.sync.dma_start(out=gs_all[:], in_=gslot_d[:])
            gv_all = mpool.tile([128, meta["M_total"]], _F32, tag="gvA")
            nc.sync.dma_start(out=gv_all[:], in_=gval_d[:])

            for c in range(N_CHUNKS):
                g_ts, s_ts = [], []
                for b in range(NB):
                    ndesc = int(T_cb[c, b]) * 128
                    g_t = gbuf.tile([128, int(T_cb[c, b]) * EL], _F32,
                                    tag=f"g{b}")
                    nc.gpsimd.dma_gather(
                        out_ap=g_t[:].rearrange("p (t f) -> p t f", f=EL),
                        in_ap=x_d[B_NODES * b:B_NODES * (b + 1)],
                        idxs_ap=gi_all[:, gi_off[c * NB + b]:
                                       gi_off[c * NB + b + 1]],
                        num_idxs=ndesc, num_idxs_reg=ndesc, elem_size=EL,
                        single_packet=False, queue_num=b,
                    )
                    g_ts.append(g_t)

                    mcb = int(M_cb[c, b])
                    s_t = sbuf_s.tile([128, mcb * TRI], _F32, tag=f"s{b}")
                    s3 = s_t[:].rearrange("p (m s) -> p m s", s=TRI)
                    mo = m_off[c * NB + b]
                    gs_b = gs_all[:, mo:mo + mcb].unsqueeze(
                        2).to_broadcast([128, mcb, TRI])
                    io_b = iota_t[:].unsqueeze(1).to_broadcast(
                        [128, mcb, TRI])
                    gv_b = gv_all[:, mo:mo + mcb].unsqueeze(
                        2).to_broadcast([128, mcb, TRI])
                    nc.vector.tensor_tensor(out=s3, in0=gs_b, in1=io_b,
                                            op=mybir.AluOpType.is_equal)
                    nc.vector.tensor_tensor(out=s3, in0=s3, in1=gv_b,
                                            op=mybir.AluOpType.mult)
                    s_ts.append(s_t)

                ps = psum.tile([TRI, TPC * D], _F32, space="PSUM", tag="ps")
                for (b, t, j, m, st, sp) in plan[c]["entries"]:
                    nc.tensor.matmul(
                        out=ps[:, D * j:D * j + D],
                        lhsT=s_ts[b][:, TRI * m:TRI * (m + 1)],
                        rhs=g_ts[b][:, EL * t:EL * t + D],
                        start=st, stop=sp,
                        skip_group_check=True,
                    )
                for j in plan[c]["missing"]:
                    # initialize untouched PSUM groups (tail chunk only)
                    nc.tensor.matmul(
                        out=ps[:, D * j:D * j + D],
                        lhsT=s_ts[0][:, :TRI],
                        rhs=g_ts[0][:, :D],
                        start=True, stop=True,
                        skip_group_check=True,
                    )

                sc = scp.tile([TRI, TPC * D], _F32, tag="sc")
                nc.scalar.copy(out=sc[:], in_=ps[:])
                nc.sync.dma_start(
                    out=out_d[RPCH * c:RPCH * (c + 1), :].rearrange(
                        "(j s) f -> s j f", s=TRI),
                    in_=sc[:].rearrange("p (j f) -> p j f", f=D),
                )
    nc.compile()
    return nc


# ===========================================================================
# Entry point
# ===========================================================================
_CACHE = {}


def _get_program(meta, repeat=1):
    key = (repeat, meta["GIW_total"], meta["M_total"])
    if key not in _CACHE:
        _CACHE[key] = build_program(meta, repeat)
    return _CACHE[key]


def _run(adj_rows, adj_cols, adj_vals, x):
    x64 = pad_x(np.ascontiguousarray(np.asarray(x), dtype=np.float32))
    in_maps, meta = prep_inputs(adj_rows, adj_cols, adj_vals)
    for m in in_maps:
        m["x64"] = x64
    nc = _get_program(meta)
    res = run_bass_kernel_spmd(nc, in_maps, core_ids=list(range(N_CORES)))
    out = np.empty((N_NODES, D), np.float32)
    for k in range(N_CORES):
        out[k * R_PER_CORE:(k + 1) * R_PER_CORE] = \
            res.results[k]["out"][:R_PER_CORE]
    return out, res, (in_maps, meta)


def kernel(adj_rows, adj_cols, adj_vals, x):
    out, _, _ = _run(adj_rows, adj_cols, adj_vals, x)
    return out


# revision 18
# speedup vs baseline: 1.5436x; 1.5436x over previous
"""GCN message passing (SpMM) on 8 Trainium2 NeuronCores.

out[r, :] = sum_{e: rows[e]==r} vals[e] * x[cols[e], :]  (N=100000, D=48,
E=1.6M, rows sorted).

Sharding: 1D row partitioning. Core k owns output rows [k*12500, (k+1)*12500)
and the contiguous edge range hitting those rows. No collectives.

Per-core algorithm (v3, unpadded tiles + 32-row windows + dense write):
  - Output rows are grouped into fixed windows of 32 consecutive rows;
    a chunk is 30 windows (960 rows); 14 chunks cover 12500 rows (padded).
  - x is padded to [100000, 64] f32 (256B rows) and split into 4 node-range
    buckets of 25000 rows so dma_gather's int16 indices can address each.
  - Per (chunk, bucket) the edges are sorted by (window, col) and cut into
    128-edge gather tiles with NO per-window padding (only the last tile of
    each (chunk, bucket) pads; whole pad tiles equalize cores).
  - Each tile emits one matmul per window its edges span (~2 avg):
    psum[32 rows of window, 48] += S^T @ G, where S [128, 32] is a masked
    selector (vals at (edge, row%32), zero for other windows' edges) built
    by DVE from per-matmul slot/val metadata via iota compare, and G is the
    gathered tile.  Spans are unioned across cores so all 8 cores share one
    program.  A window's matmuls are issued consecutively (PSUM groups
    must not interleave their start..stop accumulation).
  - PSUM [96, 10*48] holds a whole chunk (window w -> partitions
    32*(w%3)..+32, cols 48*(w//3)..+48); ACT copies it to SBUF and a plain
    strided DMA writes 960 dense rows to HBM (no dma_scatter_add).
  - Gathers run on 4 SWDGE queues (one per bucket); metadata is preloaded
    to SBUF once.
"""

import numpy as np

import concourse.bass as bass
import concourse.bacc as bacc
import concourse.mybir as mybir
import concourse.tile as tile
from concourse.bass_utils import run_bass_kernel_spmd

# ---------------- problem constants (hardcoded per the task contract) -------
N_NODES = 100000
D = 48
N_CORES = 8
R_PER_CORE = N_NODES // N_CORES  # 12500

# ---------------- kernel hyperparameters -----------------------------------
NB = 4                 # node-range buckets (int16 gather indices)
B_NODES = N_NODES // NB
W = 32                 # rows per window (= PSUM partition group)
GP = 3                 # partition groups per bank (offset 96 unusable)
WPC = 30               # windows per chunk (3 groups x 10 col blocks)
TPC = WPC // GP        # col blocks per bank (10; 10*48 f32 = 1920B <= 2KB)
RPCH = W * WPC         # rows per chunk = 960
N_CHUNKS = -(-R_PER_CORE // RPCH)          # 14
R_PAD = N_CHUNKS * RPCH                    # 13440
EL = 64                # padded x row, f32 elements (256B)
PAD_SLOT = 127.0       # slot id for pad edges (no iota(32) match)

_F32 = mybir.dt.float32
_I16 = mybir.dt.int16


def _wrap16(flat, reps=8):
    """[(n)] int16 -> [16*reps, n/16] in the 16-partition wrap, replicated."""
    n = flat.shape[0]
    w = flat.reshape(n // 16, 16).T  # [16, n/16]
    return np.tile(w, (reps, 1))


# ===========================================================================
# Host-side prep: pure index/layout transformation (no float math on data).
# ===========================================================================
def prep_inputs(adj_rows, adj_cols, adj_vals):
    """Shard + pack.  Returns (in_maps, meta) where meta drives build."""
    adj_rows = np.asarray(adj_rows).astype(np.int64)
    adj_cols = np.asarray(adj_cols).astype(np.int64)
    adj_vals = np.asarray(adj_vals).astype(np.float32)

    bounds = np.searchsorted(adj_rows, np.arange(N_CORES + 1) * R_PER_CORE)
    cores = []
    for k in range(N_CORES):
        e0, e1 = bounds[k], bounds[k + 1]
        r = adj_rows[e0:e1] - k * R_PER_CORE
        c = adj_cols[e0:e1]
        v = adj_vals[e0:e1]
        b = c // B_NODES
        cl = (c - b * B_NODES).astype(np.int64)
        win = r // W                      # global window id (0..419)
        ch = win // WPC                   # chunk id
        slot = (r % W).astype(np.float32)
        o = np.lexsort((cl, win, b * N_CHUNKS + ch))
        cores.append((b[o], ch[o], win[o], cl[o], slot[o], v[o]))

    # tiles per (chunk, bucket): max over cores
    cnt_kcb = np.zeros((N_CORES, N_CHUNKS, NB), np.int64)
    for k in range(N_CORES):
        b, ch = cores[k][0], cores[k][1]
        np.add.at(cnt_kcb[k], (ch, b), 1)
    T_cb = -(-cnt_kcb.max(axis=0) // 128)

    # per (c,b,t): union of spanned windows across cores
    spans = {}
    for k in range(N_CORES):
        b, ch, win = cores[k][0], cores[k][1], cores[k][2]
        grp = b * N_CHUNKS + ch
        gb = np.searchsorted(grp, np.arange(NB * N_CHUNKS))
        ge = np.searchsorted(grp, np.arange(NB * N_CHUNKS), side="right")
        for bb in range(NB):
            for cc in range(N_CHUNKS):
                g0, g1 = gb[bb * N_CHUNKS + cc], ge[bb * N_CHUNKS + cc]
                ww = win[g0:g1]
                for t in range((g1 - g0 + 127) // 128):
                    seg = ww[t * 128:(t + 1) * 128]
                    s = spans.setdefault((cc, bb, t), set())
                    s.update(np.unique(seg).tolist())

    # plan[c] = ordered matmuls; a window's matmuls must be consecutive
    plan = []
    M_cb = np.zeros((N_CHUNKS, NB), np.int64)
    for c in range(N_CHUNKS):
        entries = []   # (b, t, w_local, m_local)
        for b in range(NB):
            m = 0
            for t in range(T_cb[c, b]):
                ws = sorted(spans.get((c, b, t), {c * WPC}))
                for wg in ws:
                    entries.append((b, t, wg - c * WPC, m))
                    m += 1
            M_cb[c, b] = m
        entries.sort(key=lambda e: (e[2], e[0], e[1]))
        first, last = {}, {}
        for i, (b, t, w, m) in enumerate(entries):
            first.setdefault(w, i)
            last[w] = i
        missing = [w for w in range(WPC) if w not in first]
        plan.append({
            "entries": [(b, t, w, m, i == first[w], i == last[w])
                        for i, (b, t, w, m) in enumerate(entries)],
            "missing": missing,
        })

    # missing PSUM groups may only cover pad rows (sliced off by host)
    for c, p in enumerate(plan):
        for w in p["missing"]:
            assert c * RPCH + w * W >= R_PER_CORE, (c, w)

    # metadata arrays per core
    in_maps = []
    gi_w = (T_cb * 128 // 16)               # int16 cols per (c,b)
    gi_off = np.concatenate([[0], np.cumsum(gi_w.reshape(-1))])
    m_off = np.concatenate([[0], np.cumsum(M_cb.reshape(-1))])
    M_total = int(m_off[-1])
    GIW_total = int(gi_off[-1])

    iota = np.broadcast_to(np.arange(W, dtype=np.float32), (128, W)).copy()
    for k in range(N_CORES):
        b, ch, win, cl, slot, val = cores[k]
        grp = b * N_CHUNKS + ch
        gb = np.searchsorted(grp, np.arange(NB * N_CHUNKS))
        ge = np.searchsorted(grp, np.arange(NB * N_CHUNKS), side="right")
        gidx = np.zeros((128, GIW_total), np.int16)
        gslot = np.full((128, M_total), PAD_SLOT, np.float32)
        gval = np.zeros((128, M_total), np.float32)
        for c in range(N_CHUNKS):
            for bb in range(NB):
                g0, g1 = gb[bb * N_CHUNKS + c], ge[bb * N_CHUNKS + c]
                n = g1 - g0
                cap = int(T_cb[c, bb]) * 128
                idx = np.zeros(cap, np.int16)
                idx[:n] = cl[g0:g1]
                wn = np.full(cap, -1, np.int64)
                wn[:n] = win[g0:g1]
                sl = np.full(cap, PAD_SLOT, np.float32)
                sl[:n] = slot[g0:g1]
                vv = np.zeros(cap, np.float32)
                vv[:n] = val[g0:g1]
                gidx[:, gi_off[c * NB + bb]:gi_off[c * NB + bb + 1]] = \
                    _wrap16(idx)
                mo = m_off[c * NB + bb]
                for (b2, t, w, m, st, sp) in plan[c]["entries"]:
                    if b2 != bb:
                        continue
                    wg = w + c * WPC
                    seg = np.arange(t * 128, (t + 1) * 128)
                    mine = wn[seg] == wg
                    gslot[:, mo + m] = np.where(mine, sl[seg], PAD_SLOT)
                    gval[:, mo + m] = np.where(mine, vv[seg], 0.0)
        in_maps.append({"gidx": gidx, "gslot": gslot, "gval": gval,
                        "iota": iota})

    meta = {"T_cb": T_cb, "M_cb": M_cb, "gi_off": gi_off, "m_off": m_off,
            "M_total": M_total, "GIW_total": GIW_total, "plan": plan}
    return in_maps, meta


def pad_x(x):
    x64 = np.zeros((N_NODES, EL), np.float32)
    x64[:, :D] = x
    return x64


# ===========================================================================
# Device program (shared across all 8 cores)
# ===========================================================================
def build_program(meta, repeat=1, do_gather=True, do_dve=True, do_pe=True):
    T_cb = meta["T_cb"]
    M_cb = meta["M_cb"]
    gi_off = meta["gi_off"]
    m_off = meta["m_off"]
    plan = meta["plan"]

    nc = bacc.Bacc("TRN2", target_bir_lowering=False, debug=False,
                   num_devices=N_CORES, num_swdge_queues=4)
    x_d = nc.dram_tensor("x64", [N_NODES, EL], _F32, kind="ExternalInput")
    gidx_d = nc.dram_tensor("gidx", [128, meta["GIW_total"]], _I16,
                            kind="ExternalInput")
    gslot_d = nc.dram_tensor("gslot", [128, meta["M_total"]], _F32,
                             kind="ExternalInput")
    gval_d = nc.dram_tensor("gval", [128, meta["M_total"]], _F32,
                            kind="ExternalInput")
    iota_d = nc.dram_tensor("iota", [128, W], _F32, kind="ExternalInput")
    out_d = nc.dram_tensor("out", [R_PAD, D], _F32, kind="ExternalOutput")

    with tile.TileContext(nc) as tc:
        with (
            tc.tile_pool(name="meta", bufs=1) as mpool,
            tc.tile_pool(name="gbuf", bufs=2) as gbuf,
            tc.tile_pool(name="sbuf_s", bufs=1) as sbuf_s,
            tc.tile_pool(name="scp", bufs=2) as scp,
            tc.tile_pool(name="psum", bufs=2, space="PSUM") as psum,
        ):
          for _rep in range(repeat):
            iota_t = mpool.tile([128, W], _F32, tag="iota")
            nc.sync.dma_start(out=iota_t[:], in_=iota_d[:])
            gi_all = mpool.tile([128, meta["GIW_total"]], _I16, tag="giA")
            nc.sync.dma_start(out=gi_all[:], in_=gidx_d[:])
            gs_all = mpool.tile([128, meta["M_total"]], _F32, tag="gsA")
            nc.sync.dma_start(out=gs_all[:], in_=gslot_d[:])
            gv_all = mpool.tile([128, meta["M_total"]], _F32, tag="gvA")
            nc.sync.dma_start(out=gv_all[:], in_=gval_d[:])

            for c in range(N_CHUNKS):
                g_ts, s_ts = [], []
                for b in range(NB):
                    ndesc = int(T_cb[c, b]) * 128
                    g_t = gbuf.tile([128, int(T_cb[c, b]) * EL], _F32,
                                    tag=f"g{b}")
                    if do_gather:
                        nc.gpsimd.dma_gather(
                            out_ap=g_t[:].rearrange("p (t f) -> p t f", f=EL),
                            in_ap=x_d[B_NODES * b:B_NODES * (b + 1)],
                            idxs_ap=gi_all[:, gi_off[c * NB + b]:
                                           gi_off[c * NB + b + 1]],
                            num_idxs=ndesc, num_idxs_reg=ndesc, elem_size=EL,
                            single_packet=False, queue_num=b,
                        )
                    g_ts.append(g_t)

                    mcb = int(M_cb[c, b])
                    s_t = sbuf_s.tile([128, mcb * W], _F32, tag=f"s{b}")
                    if do_dve:
                        s3 = s_t[:].rearrange("p (m s) -> p m s", s=W)
                        mo = m_off[c * NB + b]
                        gs_b = gs_all[:, mo:mo + mcb].unsqueeze(
                            2).to_broadcast([128, mcb, W])
                        io_b = iota_t[:].unsqueeze(1).to_broadcast(
                            [128, mcb, W])
                        gv_b = gv_all[:, mo:mo + mcb].unsqueeze(
                            2).to_broadcast([128, mcb, W])
                        nc.vector.tensor_tensor(out=s3, in0=gs_b, in1=io_b,
                                                op=mybir.AluOpType.is_equal)
                        nc.vector.tensor_tensor(out=s3, in0=s3, in1=gv_b,
                                                op=mybir.AluOpType.mult)
                    s_ts.append(s_t)

                if do_pe and do_dve and do_gather:
                    ps = psum.tile([128, TPC * D], _F32, space="PSUM",
                                   tag="ps")
                    for (b, t, w, m, st, sp) in plan[c]["entries"]:
                        a, j = w % GP, w // GP
                        nc.tensor.matmul(
                            out=ps[32 * a:32 * a + W, D * j:D * j + D],
                            lhsT=s_ts[b][:, W * m:W * (m + 1)],
                            rhs=g_ts[b][:, EL * t:EL * t + D],
                            start=st, stop=sp,
                            skip_group_check=True,
                        )
                    for w in plan[c]["missing"]:
                        a, j = w % GP, w // GP
                        nc.tensor.matmul(
                            out=ps[32 * a:32 * a + W, D * j:D * j + D],
                            lhsT=s_ts[0][:, :W],
                            rhs=g_ts[0][:, :D],
                            start=True, stop=True,
                            skip_group_check=True,
                        )

                    sc = scp.tile([96, TPC * D], _F32, tag="sc")
                    nc.scalar.copy(out=sc[:], in_=ps[:96])
                    nc.sync.dma_start(
                        out=out_d[RPCH * c:RPCH * (c + 1), :].rearrange(
                            "(j a s) f -> (a s) j f", a=GP, s=W),
                        in_=sc[:].rearrange("p (j f) -> p j f", f=D),
                    )
    nc.compile()
    return nc


# ===========================================================================
# Entry point
# ===========================================================================
_CACHE = {}


def _get_program(meta, repeat=1):
    key = (repeat, meta["GIW_total"], meta["M_total"])
    if key not in _CACHE:
        _CACHE[key] = build_program(meta, repeat)
    return _CACHE[key]


def _run(adj_rows, adj_cols, adj_vals, x):
    x64 = pad_x(np.ascontiguousarray(np.asarray(x), dtype=np.float32))
    in_maps, meta = prep_inputs(adj_rows, adj_cols, adj_vals)
    for m in in_maps:
        m["x64"] = x64
    nc = _get_program(meta)
    res = run_bass_kernel_spmd(nc, in_maps, core_ids=list(range(N_CORES)))
    out = np.empty((N_NODES, D), np.float32)
    for k in range(N_CORES):
        out[k * R_PER_CORE:(k + 1) * R_PER_CORE] = \
            res.results[k]["out"][:R_PER_CORE]
    return out, res, (in_maps, meta)


def kernel(adj_rows, adj_cols, adj_vals, x):
    out, _, _ = _run(adj_rows, adj_cols, adj_vals, x)
    return out


# revision 21
# speedup vs baseline: 2.2162x; 1.4358x over previous
"""GCN message passing (SpMM) on 8 Trainium2 NeuronCores.

out[r, :] = sum_{e: rows[e]==r} vals[e] * x[cols[e], :]  (N=100000, D=48,
E=1.6M, rows sorted).

Sharding: 1D row partitioning. Core k owns output rows [k*12500, (k+1)*12500)
and the contiguous edge range hitting those rows. No collectives.

Per-core algorithm (v3, unpadded tiles + 32-row windows + dense write):
  - Output rows are grouped into fixed windows of 32 consecutive rows;
    a chunk is 30 windows (960 rows); 14 chunks cover 12500 rows (padded).
  - x is padded to [100000, 64] f32 (256B rows) and split into 4 node-range
    buckets of 25000 rows so dma_gather's int16 indices can address each.
  - Per (chunk, bucket) the edges are sorted by (window, col) and cut into
    128-edge gather tiles with NO per-window padding (only the last tile of
    each (chunk, bucket) pads; whole pad tiles equalize cores).
  - Each tile emits one matmul per window its edges span (~2 avg):
    psum[32 rows of window, 48] += S^T @ G, where S [128, 32] is a masked
    selector (vals at (edge, row%32), zero for other windows' edges) built
    by DVE from per-matmul slot/val metadata via iota compare, and G is the
    gathered tile.  Spans are unioned across cores so all 8 cores share one
    program.  A window's matmuls are issued consecutively (PSUM groups
    must not interleave their start..stop accumulation).
  - PSUM [96, 10*48] holds a whole chunk (window w -> partitions
    32*(w%3)..+32, cols 48*(w//3)..+48); ACT copies it to SBUF and a plain
    strided DMA writes 960 dense rows to HBM (no dma_scatter_add).
  - Gathers run on 4 SWDGE queues (one per bucket); metadata is preloaded
    to SBUF once.
"""

import numpy as np

import concourse.bass as bass
import concourse.bacc as bacc
import concourse.mybir as mybir
import concourse.tile as tile
from concourse.bass_utils import run_bass_kernel_spmd

# ---------------- problem constants (hardcoded per the task contract) -------
N_NODES = 100000
D = 48
N_CORES = 8
R_PER_CORE = N_NODES // N_CORES  # 12500

# ---------------- kernel hyperparameters -----------------------------------
NB = 4                 # node-range buckets (int16 gather indices)
B_NODES = N_NODES // NB
W = 32                 # rows per window (= PSUM partition group)
GP = 3                 # partition groups per bank (offset 96 unusable)
WPC = 30               # windows per chunk (3 groups x 10 col blocks)
TPC = WPC // GP        # col blocks per bank (10; 10*48 f32 = 1920B <= 2KB)
RPCH = W * WPC         # rows per chunk = 960
N_CHUNKS = -(-R_PER_CORE // RPCH)          # 14
R_PAD = N_CHUNKS * RPCH                    # 13440
EL = 128               # padded x row, bf16 elements (256B)
PAD_SLOT = 127.0       # slot id for pad edges (no iota(32) match)

_F32 = mybir.dt.float32
_BF16 = mybir.dt.bfloat16
_I16 = mybir.dt.int16
NP_BF16 = mybir.dt.np(mybir.dt.bfloat16)


def _wrap16(flat, reps=8):
    """[(n)] int16 -> [16*reps, n/16] in the 16-partition wrap, replicated."""
    n = flat.shape[0]
    w = flat.reshape(n // 16, 16).T  # [16, n/16]
    return np.tile(w, (reps, 1))


# ===========================================================================
# Host-side prep: pure index/layout transformation (no float math on data).
# ===========================================================================
def prep_inputs(adj_rows, adj_cols, adj_vals):
    """Shard + pack.  Returns (in_maps, meta) where meta drives build."""
    adj_rows = np.asarray(adj_rows).astype(np.int64)
    adj_cols = np.asarray(adj_cols).astype(np.int64)
    adj_vals = np.asarray(adj_vals).astype(np.float32)

    bounds = np.searchsorted(adj_rows, np.arange(N_CORES + 1) * R_PER_CORE)
    cores = []
    for k in range(N_CORES):
        e0, e1 = bounds[k], bounds[k + 1]
        r = adj_rows[e0:e1] - k * R_PER_CORE
        c = adj_cols[e0:e1]
        v = adj_vals[e0:e1]
        b = c // B_NODES
        cl = (c - b * B_NODES).astype(np.int64)
        win = r // W                      # global window id (0..419)
        ch = win // WPC                   # chunk id
        slot = (r % W).astype(np.float32)
        o = np.lexsort((cl, win, b * N_CHUNKS + ch))
        cores.append((b[o], ch[o], win[o], cl[o], slot[o], v[o]))

    # tiles per (chunk, bucket): max over cores
    cnt_kcb = np.zeros((N_CORES, N_CHUNKS, NB), np.int64)
    for k in range(N_CORES):
        b, ch = cores[k][0], cores[k][1]
        np.add.at(cnt_kcb[k], (ch, b), 1)
    T_cb = -(-cnt_kcb.max(axis=0) // 128)

    # per (c,b,t): union of spanned windows across cores
    spans = {}
    for k in range(N_CORES):
        b, ch, win = cores[k][0], cores[k][1], cores[k][2]
        grp = b * N_CHUNKS + ch
        gb = np.searchsorted(grp, np.arange(NB * N_CHUNKS))
        ge = np.searchsorted(grp, np.arange(NB * N_CHUNKS), side="right")
        for bb in range(NB):
            for cc in range(N_CHUNKS):
                g0, g1 = gb[bb * N_CHUNKS + cc], ge[bb * N_CHUNKS + cc]
                ww = win[g0:g1]
                for t in range((g1 - g0 + 127) // 128):
                    seg = ww[t * 128:(t + 1) * 128]
                    s = spans.setdefault((cc, bb, t), set())
                    s.update(np.unique(seg).tolist())

    # plan[c] = ordered matmuls; a window's matmuls must be consecutive
    plan = []
    M_cb = np.zeros((N_CHUNKS, NB), np.int64)
    for c in range(N_CHUNKS):
        entries = []   # (b, t, w_local, m_local)
        for b in range(NB):
            m = 0
            for t in range(T_cb[c, b]):
                ws = sorted(spans.get((c, b, t), {c * WPC}))
                for wg in ws:
                    entries.append((b, t, wg - c * WPC, m))
                    m += 1
            M_cb[c, b] = m
        entries.sort(key=lambda e: (e[2], e[0], e[1]))
        first, last = {}, {}
        for i, (b, t, w, m) in enumerate(entries):
            first.setdefault(w, i)
            last[w] = i
        missing = [w for w in range(WPC) if w not in first]
        plan.append({
            "entries": [(b, t, w, m, i == first[w], i == last[w])
                        for i, (b, t, w, m) in enumerate(entries)],
            "missing": missing,
        })

    # missing PSUM groups may only cover pad rows (sliced off by host)
    for c, p in enumerate(plan):
        for w in p["missing"]:
            assert c * RPCH + w * W >= R_PER_CORE, (c, w)

    # metadata arrays per core
    in_maps = []
    gi_w = (T_cb * 128 // 16)               # int16 cols per (c,b)
    gi_off = np.concatenate([[0], np.cumsum(gi_w.reshape(-1))])
    m_off = np.concatenate([[0], np.cumsum(M_cb.reshape(-1))])
    M_total = int(m_off[-1])
    GIW_total = int(gi_off[-1])

    iota = np.broadcast_to(np.arange(W, dtype=np.float32),
                       (128, W)).astype(NP_BF16)
    for k in range(N_CORES):
        b, ch, win, cl, slot, val = cores[k]
        grp = b * N_CHUNKS + ch
        gb = np.searchsorted(grp, np.arange(NB * N_CHUNKS))
        ge = np.searchsorted(grp, np.arange(NB * N_CHUNKS), side="right")
        gidx = np.zeros((128, GIW_total), np.int16)
        gslot = np.full((128, M_total), PAD_SLOT, np.float32)
        gval = np.zeros((128, M_total), np.float32)  # cast to bf16 below
        for c in range(N_CHUNKS):
            for bb in range(NB):
                g0, g1 = gb[bb * N_CHUNKS + c], ge[bb * N_CHUNKS + c]
                n = g1 - g0
                cap = int(T_cb[c, bb]) * 128
                idx = np.zeros(cap, np.int16)
                idx[:n] = cl[g0:g1]
                wn = np.full(cap, -1, np.int64)
                wn[:n] = win[g0:g1]
                sl = np.full(cap, PAD_SLOT, np.float32)
                sl[:n] = slot[g0:g1]
                vv = np.zeros(cap, np.float32)
                vv[:n] = val[g0:g1]
                gidx[:, gi_off[c * NB + bb]:gi_off[c * NB + bb + 1]] = \
                    _wrap16(idx)
                mo = m_off[c * NB + bb]
                for (b2, t, w, m, st, sp) in plan[c]["entries"]:
                    if b2 != bb:
                        continue
                    wg = w + c * WPC
                    seg = np.arange(t * 128, (t + 1) * 128)
                    mine = wn[seg] == wg
                    gslot[:, mo + m] = np.where(mine, sl[seg], PAD_SLOT)
                    gval[:, mo + m] = np.where(mine, vv[seg], 0.0)
        in_maps.append({"gidx": gidx,
                        "gslot": gslot.astype(NP_BF16),
                        "gval": gval.astype(NP_BF16),
                        "iota": iota})

    meta = {"T_cb": T_cb, "M_cb": M_cb, "gi_off": gi_off, "m_off": m_off,
            "M_total": M_total, "GIW_total": GIW_total, "plan": plan}
    return in_maps, meta


def pad_x(x):
    x64 = np.zeros((N_NODES, EL), NP_BF16)
    x64[:, :D] = np.asarray(x, np.float32).astype(NP_BF16)
    return x64


# ===========================================================================
# Device program (shared across all 8 cores)
# ===========================================================================
def build_program(meta, repeat=1, do_gather=True, do_dve=True, do_pe=True,
                  dve_half=False):
    T_cb = meta["T_cb"]
    M_cb = meta["M_cb"]
    gi_off = meta["gi_off"]
    m_off = meta["m_off"]
    plan = meta["plan"]

    nc = bacc.Bacc("TRN2", target_bir_lowering=False, debug=False,
                   num_devices=N_CORES, num_swdge_queues=4)
    x_d = nc.dram_tensor("x64", [N_NODES, EL], _BF16, kind="ExternalInput")
    gidx_d = nc.dram_tensor("gidx", [128, meta["GIW_total"]], _I16,
                            kind="ExternalInput")
    gslot_d = nc.dram_tensor("gslot", [128, meta["M_total"]], _BF16,
                             kind="ExternalInput")
    gval_d = nc.dram_tensor("gval", [128, meta["M_total"]], _BF16,
                            kind="ExternalInput")
    iota_d = nc.dram_tensor("iota", [128, W], _BF16, kind="ExternalInput")
    out_d = nc.dram_tensor("out", [R_PAD, D], _F32, kind="ExternalOutput")

    with tile.TileContext(nc) as tc:
        with (
            tc.tile_pool(name="meta", bufs=1) as mpool,
            tc.tile_pool(name="gbuf", bufs=2) as gbuf,
            tc.tile_pool(name="sbuf_s", bufs=2) as sbuf_s,
            tc.tile_pool(name="scp", bufs=2) as scp,
            tc.tile_pool(name="psum", bufs=2, space="PSUM") as psum,
        ):
          for _rep in range(repeat):
            iota_t = mpool.tile([128, W], _BF16, tag="iota")
            nc.sync.dma_start(out=iota_t[:], in_=iota_d[:])
            gi_all = mpool.tile([128, meta["GIW_total"]], _I16, tag="giA")
            nc.sync.dma_start(out=gi_all[:], in_=gidx_d[:])
            gs_all = mpool.tile([128, meta["M_total"]], _BF16, tag="gsA")
            nc.sync.dma_start(out=gs_all[:], in_=gslot_d[:])
            gv_all = mpool.tile([128, meta["M_total"]], _BF16, tag="gvA")
            nc.sync.dma_start(out=gv_all[:], in_=gval_d[:])

            for c in range(N_CHUNKS):
                g_ts, s_ts = [], []
                for b in range(NB):
                    ndesc = int(T_cb[c, b]) * 128
                    g_t = gbuf.tile([128, int(T_cb[c, b]) * EL], _BF16,
                                    tag=f"g{b}")
                    if do_gather:
                        nc.gpsimd.dma_gather(
                            out_ap=g_t[:].rearrange("p (t f) -> p t f", f=EL),
                            in_ap=x_d[B_NODES * b:B_NODES * (b + 1)],
                            idxs_ap=gi_all[:, gi_off[c * NB + b]:
                                           gi_off[c * NB + b + 1]],
                            num_idxs=ndesc, num_idxs_reg=ndesc, elem_size=EL,
                            single_packet=False, queue_num=b,
                        )
                    g_ts.append(g_t)

                    mcb = int(M_cb[c, b])
                    s_t = sbuf_s.tile([128, mcb * W], _BF16, tag=f"s{b}")
                    if do_dve:
                        s3 = s_t[:].rearrange("p (m s) -> p m s", s=W)
                        mo = m_off[c * NB + b]
                        gs_b = gs_all[:, mo:mo + mcb].unsqueeze(
                            2).to_broadcast([128, mcb, W])
                        io_b = iota_t[:].unsqueeze(1).to_broadcast(
                            [128, mcb, W])
                        gv_b = gv_all[:, mo:mo + mcb].unsqueeze(
                            2).to_broadcast([128, mcb, W])
                        nc.vector.tensor_tensor(out=s3, in0=gs_b, in1=io_b,
                                                op=mybir.AluOpType.is_equal)
                        if not dve_half:
                            nc.vector.tensor_tensor(out=s3, in0=s3, in1=gv_b,
                                                    op=mybir.AluOpType.mult)
                    s_ts.append(s_t)

                if do_pe and do_dve and do_gather:
                    ps = psum.tile([128, TPC * D], _F32, space="PSUM",
                                   tag="ps")
                    for (b, t, w, m, st, sp) in plan[c]["entries"]:
                        a, j = w % GP, w // GP
                        nc.tensor.matmul(
                            out=ps[32 * a:32 * a + W, D * j:D * j + D],
                            lhsT=s_ts[b][:, W * m:W * (m + 1)],
                            rhs=g_ts[b][:, EL * t:EL * t + D],
                            start=st, stop=sp,
                            skip_group_check=True,
                        )
                    for w in plan[c]["missing"]:
                        a, j = w % GP, w // GP
                        nc.tensor.matmul(
                            out=ps[32 * a:32 * a + W, D * j:D * j + D],
                            lhsT=s_ts[0][:, :W],
                            rhs=g_ts[0][:, :D],
                            start=True, stop=True,
                            skip_group_check=True,
                        )

                    sc = scp.tile([96, TPC * D], _F32, tag="sc")
                    nc.scalar.copy(out=sc[:], in_=ps[:96])
                    nc.sync.dma_start(
                        out=out_d[RPCH * c:RPCH * (c + 1), :].rearrange(
                            "(j a s) f -> (a s) j f", a=GP, s=W),
                        in_=sc[:].rearrange("p (j f) -> p j f", f=D),
                    )
    nc.compile()
    return nc


# ===========================================================================
# Entry point
# ===========================================================================
_CACHE = {}


def _get_program(meta, repeat=1):
    key = (repeat, meta["GIW_total"], meta["M_total"])
    if key not in _CACHE:
        _CACHE[key] = build_program(meta, repeat)
    return _CACHE[key]


def _run(adj_rows, adj_cols, adj_vals, x):
    x64 = pad_x(np.ascontiguousarray(np.asarray(x), dtype=np.float32))
    in_maps, meta = prep_inputs(adj_rows, adj_cols, adj_vals)
    for m in in_maps:
        m["x64"] = x64
    nc = _get_program(meta)
    res = run_bass_kernel_spmd(nc, in_maps, core_ids=list(range(N_CORES)))
    out = np.empty((N_NODES, D), np.float32)
    for k in range(N_CORES):
        out[k * R_PER_CORE:(k + 1) * R_PER_CORE] = \
            res.results[k]["out"][:R_PER_CORE]
    return out, res, (in_maps, meta)


def kernel(adj_rows, adj_cols, adj_vals, x):
    out, _, _ = _run(adj_rows, adj_cols, adj_vals, x)
    return out


# revision 24
# speedup vs baseline: 2.8429x; 1.2828x over previous
"""GCN message passing (SpMM) on 8 Trainium2 NeuronCores.

out[r, :] = sum_{e: rows[e]==r} vals[e] * x[cols[e], :]  (N=100000, D=48,
E=1.6M, rows sorted).

Sharding: 1D row partitioning. Core k owns output rows [k*12500, (k+1)*12500)
and the contiguous edge range hitting those rows. No collectives.

Per-core algorithm (v3, unpadded tiles + 32-row windows + dense write):
  - Output rows are grouped into fixed windows of 32 consecutive rows;
    a chunk is 30 windows (960 rows); 14 chunks cover 12500 rows (padded).
  - x is padded to [100000, 64] f32 (256B rows) and split into 4 node-range
    buckets of 25000 rows so dma_gather's int16 indices can address each.
  - Per (chunk, bucket) the edges are sorted by (window, col) and cut into
    128-edge gather tiles with NO per-window padding (only the last tile of
    each (chunk, bucket) pads; whole pad tiles equalize cores).
  - Each tile emits one matmul per window its edges span (~2 avg):
    psum[32 rows of window, 48] += S^T @ G, where S [128, 32] is a masked
    selector (vals at (edge, row%32), zero for other windows' edges) built
    by DVE from per-matmul slot/val metadata via iota compare, and G is the
    gathered tile.  Spans are unioned across cores so all 8 cores share one
    program.  A window's matmuls are issued consecutively (PSUM groups
    must not interleave their start..stop accumulation).
  - PSUM [96, 10*48] holds a whole chunk (window w -> partitions
    32*(w%3)..+32, cols 48*(w//3)..+48); ACT copies it to SBUF and a plain
    strided DMA writes 960 dense rows to HBM (no dma_scatter_add).
  - Gathers run on 4 SWDGE queues (one per bucket); metadata is preloaded
    to SBUF once.
"""

import numpy as np

import concourse.bass as bass
import concourse.bacc as bacc
import concourse.mybir as mybir
import concourse.tile as tile
from concourse.bass_utils import run_bass_kernel_spmd

# ---------------- problem constants (hardcoded per the task contract) -------
N_NODES = 100000
D = 48
N_CORES = 8
R_PER_CORE = N_NODES // N_CORES  # 12500

# ---------------- kernel hyperparameters -----------------------------------
NB = 4                 # node-range buckets (int16 gather indices)
B_NODES = N_NODES // NB
W = 32                 # rows per window (= PSUM partition group)
GP = 3                 # partition groups per bank (offset 96 unusable)
WPC = 30               # windows per chunk (3 groups x 10 col blocks)
TPC = WPC // GP        # col blocks per bank (10; 10*48 f32 = 1920B <= 2KB)
RPCH = W * WPC         # rows per chunk = 960
N_CHUNKS = -(-R_PER_CORE // RPCH)          # 14
R_PAD = N_CHUNKS * RPCH                    # 13440
EL = 128               # padded x row, bf16 elements (256B)
PAD_SLOT = 127.0       # slot id for pad edges (no iota(32) match)

_F32 = mybir.dt.float32
_BF16 = mybir.dt.bfloat16
_I16 = mybir.dt.int16
NP_BF16 = mybir.dt.np(mybir.dt.bfloat16)


def _wrap16(flat, reps=8):
    """[(n)] int16 -> [16*reps, n/16] in the 16-partition wrap, replicated."""
    n = flat.shape[0]
    w = flat.reshape(n // 16, 16).T  # [16, n/16]
    return np.tile(w, (reps, 1))


# ===========================================================================
# Host-side prep: pure index/layout transformation (no float math on data).
# ===========================================================================
def prep_inputs(adj_rows, adj_cols, adj_vals):
    """Shard + pack.  Returns (in_maps, meta) where meta drives build."""
    adj_rows = np.asarray(adj_rows).astype(np.int64)
    adj_cols = np.asarray(adj_cols).astype(np.int64)
    adj_vals = np.asarray(adj_vals).astype(np.float32)

    bounds = np.searchsorted(adj_rows, np.arange(N_CORES + 1) * R_PER_CORE)
    cores = []
    for k in range(N_CORES):
        e0, e1 = bounds[k], bounds[k + 1]
        r = adj_rows[e0:e1] - k * R_PER_CORE
        c = adj_cols[e0:e1]
        v = adj_vals[e0:e1]
        b = c // B_NODES
        cl = (c - b * B_NODES).astype(np.int64)
        win = r // W                      # global window id (0..419)
        ch = win // WPC                   # chunk id
        slot = (r % W).astype(np.float32)
        o = np.lexsort((cl, win, b * N_CHUNKS + ch))
        cores.append((b[o], ch[o], win[o], cl[o], slot[o], v[o]))

    # tiles per (chunk, bucket): max over cores
    cnt_kcb = np.zeros((N_CORES, N_CHUNKS, NB), np.int64)
    for k in range(N_CORES):
        b, ch = cores[k][0], cores[k][1]
        np.add.at(cnt_kcb[k], (ch, b), 1)
    T_cb = -(-cnt_kcb.max(axis=0) // 128)

    # per (c,b,t): union of spanned windows across cores
    spans = {}
    for k in range(N_CORES):
        b, ch, win = cores[k][0], cores[k][1], cores[k][2]
        grp = b * N_CHUNKS + ch
        gb = np.searchsorted(grp, np.arange(NB * N_CHUNKS))
        ge = np.searchsorted(grp, np.arange(NB * N_CHUNKS), side="right")
        for bb in range(NB):
            for cc in range(N_CHUNKS):
                g0, g1 = gb[bb * N_CHUNKS + cc], ge[bb * N_CHUNKS + cc]
                ww = win[g0:g1]
                for t in range((g1 - g0 + 127) // 128):
                    seg = ww[t * 128:(t + 1) * 128]
                    s = spans.setdefault((cc, bb, t), set())
                    s.update(np.unique(seg).tolist())

    # plan[c] = ordered matmuls; a window's matmuls must be consecutive
    plan = []
    M_cb = np.zeros((N_CHUNKS, NB), np.int64)
    for c in range(N_CHUNKS):
        entries = []   # (b, t, w_local, m_local)
        for b in range(NB):
            m = 0
            for t in range(T_cb[c, b]):
                ws = sorted(spans.get((c, b, t), {c * WPC}))
                for wg in ws:
                    entries.append((b, t, wg - c * WPC, m))
                    m += 1
            M_cb[c, b] = m
        entries.sort(key=lambda e: (e[2], e[0], e[1]))
        first, last = {}, {}
        for i, (b, t, w, m) in enumerate(entries):
            first.setdefault(w, i)
            last[w] = i
        missing = [w for w in range(WPC) if w not in first]
        plan.append({
            "entries": [(b, t, w, m, i == first[w], i == last[w])
                        for i, (b, t, w, m) in enumerate(entries)],
            "missing": missing,
        })

    # missing PSUM groups may only cover pad rows (sliced off by host)
    for c, p in enumerate(plan):
        for w in p["missing"]:
            assert c * RPCH + w * W >= R_PER_CORE, (c, w)

    # metadata arrays per core
    in_maps = []
    gi_w = (T_cb * 128 // 16)               # int16 cols per (c,b)
    gi_off = np.concatenate([[0], np.cumsum(gi_w.reshape(-1))])
    m_off = np.concatenate([[0], np.cumsum(M_cb.reshape(-1))])
    M_total = int(m_off[-1])
    GIW_total = int(gi_off[-1])

    iota = np.broadcast_to(np.arange(W, dtype=np.float32),
                       (128, W)).astype(NP_BF16)
    for k in range(N_CORES):
        b, ch, win, cl, slot, val = cores[k]
        grp = b * N_CHUNKS + ch
        gb = np.searchsorted(grp, np.arange(NB * N_CHUNKS))
        ge = np.searchsorted(grp, np.arange(NB * N_CHUNKS), side="right")
        gidx = np.zeros((128, GIW_total), np.int16)
        gslot = np.full((128, M_total), PAD_SLOT, np.float32)
        gval = np.zeros((128, M_total), np.float32)  # cast to bf16 below
        for c in range(N_CHUNKS):
            for bb in range(NB):
                g0, g1 = gb[bb * N_CHUNKS + c], ge[bb * N_CHUNKS + c]
                n = g1 - g0
                cap = int(T_cb[c, bb]) * 128
                idx = np.zeros(cap, np.int16)
                idx[:n] = cl[g0:g1]
                wn = np.full(cap, -1, np.int64)
                wn[:n] = win[g0:g1]
                sl = np.full(cap, PAD_SLOT, np.float32)
                sl[:n] = slot[g0:g1]
                vv = np.zeros(cap, np.float32)
                vv[:n] = val[g0:g1]
                gidx[:, gi_off[c * NB + bb]:gi_off[c * NB + bb + 1]] = \
                    _wrap16(idx)
                mo = m_off[c * NB + bb]
                for (b2, t, w, m, st, sp) in plan[c]["entries"]:
                    if b2 != bb:
                        continue
                    wg = w + c * WPC
                    seg = np.arange(t * 128, (t + 1) * 128)
                    mine = wn[seg] == wg
                    gslot[:, mo + m] = np.where(mine, sl[seg], PAD_SLOT)
                    gval[:, mo + m] = np.where(mine, vv[seg], 0.0)
        in_maps.append({"gidx": gidx,
                        "gslot": gslot.astype(NP_BF16),
                        "gval": gval.astype(NP_BF16),
                        "iota": iota})

    meta = {"T_cb": T_cb, "M_cb": M_cb, "gi_off": gi_off, "m_off": m_off,
            "M_total": M_total, "GIW_total": GIW_total, "plan": plan}
    return in_maps, meta


def pad_x(x):
    x64 = np.zeros((N_NODES, EL), NP_BF16)
    x64[:, :D] = np.asarray(x, np.float32).astype(NP_BF16)
    return x64


# ===========================================================================
# Device program (shared across all 8 cores)
# ===========================================================================
def build_program(meta, repeat=1, do_gather=True, do_dve=True, do_pe=True,
                  dve_half=False, gbufs=3, pbufs=3):
    T_cb = meta["T_cb"]
    M_cb = meta["M_cb"]
    gi_off = meta["gi_off"]
    m_off = meta["m_off"]
    plan = meta["plan"]

    nc = bacc.Bacc("TRN2", target_bir_lowering=False, debug=False,
                   num_devices=N_CORES, num_swdge_queues=4)
    x_d = nc.dram_tensor("x64", [N_NODES, EL], _BF16, kind="ExternalInput")
    gidx_d = nc.dram_tensor("gidx", [128, meta["GIW_total"]], _I16,
                            kind="ExternalInput")
    gslot_d = nc.dram_tensor("gslot", [128, meta["M_total"]], _BF16,
                             kind="ExternalInput")
    gval_d = nc.dram_tensor("gval", [128, meta["M_total"]], _BF16,
                            kind="ExternalInput")
    iota_d = nc.dram_tensor("iota", [128, W], _BF16, kind="ExternalInput")
    out_d = nc.dram_tensor("out", [R_PAD, D], _F32, kind="ExternalOutput")

    with tile.TileContext(nc) as tc:
        with (
            tc.tile_pool(name="meta", bufs=1) as mpool,
            tc.tile_pool(name="gbuf", bufs=gbufs) as gbuf,
            tc.tile_pool(name="sbuf_s", bufs=2) as sbuf_s,
            tc.tile_pool(name="scp", bufs=2) as scp,
            tc.tile_pool(name="psum", bufs=pbufs, space="PSUM") as psum,
        ):
          for _rep in range(repeat):
            iota_t = mpool.tile([128, W], _BF16, tag="iota")
            nc.sync.dma_start(out=iota_t[:], in_=iota_d[:])
            gi_all = mpool.tile([128, meta["GIW_total"]], _I16, tag="giA")
            nc.sync.dma_start(out=gi_all[:], in_=gidx_d[:])
            gs_all = mpool.tile([128, meta["M_total"]], _BF16, tag="gsA")
            nc.sync.dma_start(out=gs_all[:], in_=gslot_d[:])
            gv_all = mpool.tile([128, meta["M_total"]], _BF16, tag="gvA")
            nc.sync.dma_start(out=gv_all[:], in_=gval_d[:])

            for c in range(N_CHUNKS):
                g_ts, s_ts = [], []
                for b in range(NB):
                    ndesc = int(T_cb[c, b]) * 128
                    g_t = gbuf.tile([128, int(T_cb[c, b]) * EL], _BF16,
                                    tag=f"g{b}")
                    if do_gather:
                        nc.gpsimd.dma_gather(
                            out_ap=g_t[:].rearrange("p (t f) -> p t f", f=EL),
                            in_ap=x_d[B_NODES * b:B_NODES * (b + 1)],
                            idxs_ap=gi_all[:, gi_off[c * NB + b]:
                                           gi_off[c * NB + b + 1]],
                            num_idxs=ndesc, num_idxs_reg=ndesc, elem_size=EL,
                            single_packet=False, queue_num=b,
                        )
                    g_ts.append(g_t)

                    mcb = int(M_cb[c, b])
                    s_t = sbuf_s.tile([128, mcb * W], _BF16, tag=f"s{b}")
                    if do_dve:
                        s3 = s_t[:].rearrange("p (m s) -> p m s", s=W)
                        mo = m_off[c * NB + b]
                        gs_b = gs_all[:, mo:mo + mcb].unsqueeze(
                            2).to_broadcast([128, mcb, W])
                        io_b = iota_t[:].unsqueeze(1).to_broadcast(
                            [128, mcb, W])
                        gv_b = gv_all[:, mo:mo + mcb].unsqueeze(
                            2).to_broadcast([128, mcb, W])
                        nc.vector.tensor_tensor(out=s3, in0=gs_b, in1=io_b,
                                                op=mybir.AluOpType.is_equal)
                        if not dve_half:
                            nc.vector.tensor_tensor(out=s3, in0=s3, in1=gv_b,
                                                    op=mybir.AluOpType.mult)
                    s_ts.append(s_t)

                if do_pe and do_dve and do_gather:
                    ps = psum.tile([128, TPC * D], _F32, space="PSUM",
                                   tag="ps")
                    for (b, t, w, m, st, sp) in plan[c]["entries"]:
                        a, j = w % GP, w // GP
                        nc.tensor.matmul(
                            out=ps[32 * a:32 * a + W, D * j:D * j + D],
                            lhsT=s_ts[b][:, W * m:W * (m + 1)],
                            rhs=g_ts[b][:, EL * t:EL * t + D],
                            start=st, stop=sp,
                            skip_group_check=True,
                        )
                    for w in plan[c]["missing"]:
                        a, j = w % GP, w // GP
                        nc.tensor.matmul(
                            out=ps[32 * a:32 * a + W, D * j:D * j + D],
                            lhsT=s_ts[0][:, :W],
                            rhs=g_ts[0][:, :D],
                            start=True, stop=True,
                            skip_group_check=True,
                        )

                    sc = scp.tile([96, TPC * D], _F32, tag="sc")
                    nc.scalar.copy(out=sc[:], in_=ps[:96])
                    nc.sync.dma_start(
                        out=out_d[RPCH * c:RPCH * (c + 1), :].rearrange(
                            "(j a s) f -> (a s) j f", a=GP, s=W),
                        in_=sc[:].rearrange("p (j f) -> p j f", f=D),
                    )
    nc.compile()
    return nc


# ===========================================================================
# Entry point
# ===========================================================================
_CACHE = {}


def _get_program(meta, repeat=1):
    key = (repeat, meta["GIW_total"], meta["M_total"])
    if key not in _CACHE:
        _CACHE[key] = build_program(meta, repeat)
    return _CACHE[key]


def _run(adj_rows, adj_cols, adj_vals, x):
    x64 = pad_x(np.ascontiguousarray(np.asarray(x), dtype=np.float32))
    in_maps, meta = prep_inputs(adj_rows, adj_cols, adj_vals)
    for m in in_maps:
        m["x64"] = x64
    nc = _get_program(meta)
    res = run_bass_kernel_spmd(nc, in_maps, core_ids=list(range(N_CORES)))
    out = np.empty((N_NODES, D), np.float32)
    for k in range(N_CORES):
        out[k * R_PER_CORE:(k + 1) * R_PER_CORE] = \
            res.results[k]["out"][:R_PER_CORE]
    return out, res, (in_maps, meta)


def kernel(adj_rows, adj_cols, adj_vals, x):
    out, _, _ = _run(adj_rows, adj_cols, adj_vals, x)
    return out


# revision 28
# speedup vs baseline: 2.8867x; 1.0154x over previous
"""GCN message passing (SpMM) on 8 Trainium2 NeuronCores.

out[r, :] = sum_{e: rows[e]==r} vals[e] * x[cols[e], :]  (N=100000, D=48,
E=1.6M, rows sorted).

Sharding: 1D row partitioning. Core k owns output rows [k*12500, (k+1)*12500)
and the contiguous edge range hitting those rows. No collectives.

Per-core algorithm (v3, unpadded tiles + 32-row windows + dense write):
  - Output rows are grouped into fixed windows of 32 consecutive rows;
    a chunk is 30 windows (960 rows); 14 chunks cover 12500 rows (padded).
  - x is padded to [100000, 64] f32 (256B rows) and split into 4 node-range
    buckets of 25000 rows so dma_gather's int16 indices can address each.
  - Per (chunk, bucket) the edges are sorted by (window, col) and cut into
    128-edge gather tiles with NO per-window padding (only the last tile of
    each (chunk, bucket) pads; whole pad tiles equalize cores).
  - Each tile emits one matmul per window its edges span (~2 avg):
    psum[32 rows of window, 48] += S^T @ G, where S [128, 32] is a masked
    selector (vals at (edge, row%32), zero for other windows' edges) built
    by DVE from per-matmul slot/val metadata via iota compare, and G is the
    gathered tile.  Spans are unioned across cores so all 8 cores share one
    program.  A window's matmuls are issued consecutively (PSUM groups
    must not interleave their start..stop accumulation).
  - PSUM [96, 10*48] holds a whole chunk (window w -> partitions
    32*(w%3)..+32, cols 48*(w//3)..+48); ACT copies it to SBUF and a plain
    strided DMA writes 960 dense rows to HBM (no dma_scatter_add).
  - Gathers run on 4 SWDGE queues (one per bucket); metadata is preloaded
    to SBUF once.
"""

import numpy as np

import concourse.bass as bass
import concourse.bacc as bacc
import concourse.mybir as mybir
import concourse.tile as tile
from concourse.bass_utils import run_bass_kernel_spmd

# ---------------- problem constants (hardcoded per the task contract) -------
N_NODES = 100000
D = 48
N_CORES = 8
R_PER_CORE = N_NODES // N_CORES  # 12500

# ---------------- kernel hyperparameters -----------------------------------
NB = 4                 # node-range buckets (int16 gather indices)
B_NODES = N_NODES // NB
W = 32                 # rows per window (= PSUM partition group)
GP = 3                 # partition groups per bank (offset 96 unusable)
WPC = 30               # windows per chunk (3 groups x 10 col blocks)
TPC = WPC // GP        # col blocks per bank (10; 10*48 f32 = 1920B <= 2KB)
RPCH = W * WPC         # rows per chunk = 960
N_CHUNKS = -(-R_PER_CORE // RPCH)          # 14
R_PAD = N_CHUNKS * RPCH                    # 13440
EL = 128               # padded x row, bf16 elements (256B)
PAD_SLOT = 127.0       # slot id for pad edges (no iota(32) match)

_F32 = mybir.dt.float32
_BF16 = mybir.dt.bfloat16
_I16 = mybir.dt.int16
NP_BF16 = mybir.dt.np(mybir.dt.bfloat16)


def _wrap16(flat, reps=8):
    """[(n)] int16 -> [16*reps, n/16] in the 16-partition wrap, replicated."""
    n = flat.shape[0]
    w = flat.reshape(n // 16, 16).T  # [16, n/16]
    return np.tile(w, (reps, 1))


# ===========================================================================
# Host-side prep: pure index/layout transformation (no float math on data).
# ===========================================================================
def prep_inputs(adj_rows, adj_cols, adj_vals):
    """Shard + pack.  Returns (in_maps, meta) where meta drives build."""
    adj_rows = np.asarray(adj_rows).astype(np.int64)
    adj_cols = np.asarray(adj_cols).astype(np.int64)
    adj_vals = np.asarray(adj_vals).astype(np.float32)

    bounds = np.searchsorted(adj_rows, np.arange(N_CORES + 1) * R_PER_CORE)
    cores = []
    for k in range(N_CORES):
        e0, e1 = bounds[k], bounds[k + 1]
        r = adj_rows[e0:e1] - k * R_PER_CORE
        c = adj_cols[e0:e1]
        v = adj_vals[e0:e1]
        b = c // B_NODES
        cl = (c - b * B_NODES).astype(np.int64)
        win = r // W                      # global window id (0..419)
        ch = win // WPC                   # chunk id
        slot = (r % W).astype(np.float32)
        o = np.lexsort((cl, win, b * N_CHUNKS + ch))
        cores.append((b[o], ch[o], win[o], cl[o], slot[o], v[o]))

    # tiles per (chunk, bucket): max over cores
    cnt_kcb = np.zeros((N_CORES, N_CHUNKS, NB), np.int64)
    for k in range(N_CORES):
        b, ch = cores[k][0], cores[k][1]
        np.add.at(cnt_kcb[k], (ch, b), 1)
    T_cb = -(-cnt_kcb.max(axis=0) // 128)

    # per (c,b,t): union of spanned windows across cores
    spans = {}
    for k in range(N_CORES):
        b, ch, win = cores[k][0], cores[k][1], cores[k][2]
        grp = b * N_CHUNKS + ch
        gb = np.searchsorted(grp, np.arange(NB * N_CHUNKS))
        ge = np.searchsorted(grp, np.arange(NB * N_CHUNKS), side="right")
        for bb in range(NB):
            for cc in range(N_CHUNKS):
                g0, g1 = gb[bb * N_CHUNKS + cc], ge[bb * N_CHUNKS + cc]
                ww = win[g0:g1]
                for t in range((g1 - g0 + 127) // 128):
                    seg = ww[t * 128:(t + 1) * 128]
                    s = spans.setdefault((cc, bb, t), set())
                    s.update(np.unique(seg).tolist())

    # plan[c] = ordered matmuls; a window's matmuls must be consecutive
    plan = []
    M_cb = np.zeros((N_CHUNKS, NB), np.int64)
    for c in range(N_CHUNKS):
        entries = []   # (b, t, w_local, m_local)
        for b in range(NB):
            m = 0
            for t in range(T_cb[c, b]):
                ws = sorted(spans.get((c, b, t), {c * WPC}))
                for wg in ws:
                    entries.append((b, t, wg - c * WPC, m))
                    m += 1
            M_cb[c, b] = m
        entries.sort(key=lambda e: (e[2], e[0], e[1]))
        first, last = {}, {}
        for i, (b, t, w, m) in enumerate(entries):
            first.setdefault(w, i)
            last[w] = i
        missing = [w for w in range(WPC) if w not in first]
        plan.append({
            "entries": [(b, t, w, m, i == first[w], i == last[w])
                        for i, (b, t, w, m) in enumerate(entries)],
            "missing": missing,
        })

    # missing PSUM groups may only cover pad rows (sliced off by host)
    for c, p in enumerate(plan):
        for w in p["missing"]:
            assert c * RPCH + w * W >= R_PER_CORE, (c, w)

    # metadata arrays per core
    in_maps = []
    gi_w = (T_cb * 128 // 16)               # int16 cols per (c,b)
    gi_off = np.concatenate([[0], np.cumsum(gi_w.reshape(-1))])
    m_off = np.concatenate([[0], np.cumsum(M_cb.reshape(-1))])
    M_total = int(m_off[-1])
    GIW_total = int(gi_off[-1])

    iota = np.broadcast_to(np.arange(W, dtype=np.float32),
                       (128, W)).astype(NP_BF16)
    for k in range(N_CORES):
        b, ch, win, cl, slot, val = cores[k]
        grp = b * N_CHUNKS + ch
        gb = np.searchsorted(grp, np.arange(NB * N_CHUNKS))
        ge = np.searchsorted(grp, np.arange(NB * N_CHUNKS), side="right")
        gidx = np.zeros((128, GIW_total), np.int16)
        gslot = np.full((128, M_total), PAD_SLOT, np.float32)
        gval = np.zeros((128, M_total), np.float32)  # cast to bf16 below
        for c in range(N_CHUNKS):
            for bb in range(NB):
                g0, g1 = gb[bb * N_CHUNKS + c], ge[bb * N_CHUNKS + c]
                n = g1 - g0
                cap = int(T_cb[c, bb]) * 128
                idx = np.zeros(cap, np.int16)
                idx[:n] = cl[g0:g1]
                wn = np.full(cap, -1, np.int64)
                wn[:n] = win[g0:g1]
                sl = np.full(cap, PAD_SLOT, np.float32)
                sl[:n] = slot[g0:g1]
                vv = np.zeros(cap, np.float32)
                vv[:n] = val[g0:g1]
                gidx[:, gi_off[c * NB + bb]:gi_off[c * NB + bb + 1]] = \
                    _wrap16(idx)
                mo = m_off[c * NB + bb]
                for (b2, t, w, m, st, sp) in plan[c]["entries"]:
                    if b2 != bb:
                        continue
                    wg = w + c * WPC
                    seg = np.arange(t * 128, (t + 1) * 128)
                    mine = wn[seg] == wg
                    gslot[:, mo + m] = np.where(mine, sl[seg], PAD_SLOT)
                    gval[:, mo + m] = np.where(mine, vv[seg], 0.0)
        in_maps.append({"gidx": gidx,
                        "gslot": gslot.astype(NP_BF16),
                        "gval": gval.astype(NP_BF16),
                        "iota": iota})

    meta = {"T_cb": T_cb, "M_cb": M_cb, "gi_off": gi_off, "m_off": m_off,
            "M_total": M_total, "GIW_total": GIW_total, "plan": plan}
    return in_maps, meta


def pad_x(x):
    x64 = np.zeros((N_NODES, EL), NP_BF16)
    x64[:, :D] = np.asarray(x, np.float32).astype(NP_BF16)
    return x64


# ===========================================================================
# Device program (shared across all 8 cores)
# ===========================================================================
def build_program(meta, repeat=1, do_gather=True, do_dve=True, do_pe=True,
                  dve_half=False, gbufs=3, pbufs=4, scbufs=3,
                  out_engine="scalar"):
    T_cb = meta["T_cb"]
    M_cb = meta["M_cb"]
    gi_off = meta["gi_off"]
    m_off = meta["m_off"]
    plan = meta["plan"]

    nc = bacc.Bacc("TRN2", target_bir_lowering=False, debug=False,
                   num_devices=N_CORES, num_swdge_queues=4)
    x_d = nc.dram_tensor("x64", [N_NODES, EL], _BF16, kind="ExternalInput")
    gidx_d = nc.dram_tensor("gidx", [128, meta["GIW_total"]], _I16,
                            kind="ExternalInput")
    gslot_d = nc.dram_tensor("gslot", [128, meta["M_total"]], _BF16,
                             kind="ExternalInput")
    gval_d = nc.dram_tensor("gval", [128, meta["M_total"]], _BF16,
                            kind="ExternalInput")
    iota_d = nc.dram_tensor("iota", [128, W], _BF16, kind="ExternalInput")
    out_d = nc.dram_tensor("out", [R_PAD, D], _F32, kind="ExternalOutput")

    with tile.TileContext(nc) as tc:
        with (
            tc.tile_pool(name="meta", bufs=1) as mpool,
            tc.tile_pool(name="gbuf", bufs=gbufs) as gbuf,
            tc.tile_pool(name="sbuf_s", bufs=2) as sbuf_s,
            tc.tile_pool(name="scp", bufs=scbufs) as scp,
            tc.tile_pool(name="psum", bufs=pbufs, space="PSUM") as psum,
        ):
          for _rep in range(repeat):
            iota_t = mpool.tile([128, W], _BF16, tag="iota")
            nc.sync.dma_start(out=iota_t[:], in_=iota_d[:])
            gi_all = mpool.tile([128, meta["GIW_total"]], _I16, tag="giA")
            nc.sync.dma_start(out=gi_all[:], in_=gidx_d[:])
            gs_all = mpool.tile([128, meta["M_total"]], _BF16, tag="gsA")
            nc.sync.dma_start(out=gs_all[:], in_=gslot_d[:])
            gv_all = mpool.tile([128, meta["M_total"]], _BF16, tag="gvA")
            nc.sync.dma_start(out=gv_all[:], in_=gval_d[:])

            for c in range(N_CHUNKS):
                g_ts, s_ts = [], []
                for b in range(NB):
                    ndesc = int(T_cb[c, b]) * 128
                    g_t = gbuf.tile([128, int(T_cb[c, b]) * EL], _BF16,
                                    tag=f"g{b}")
                    if do_gather:
                        nc.gpsimd.dma_gather(
                            out_ap=g_t[:].rearrange("p (t f) -> p t f", f=EL),
                            in_ap=x_d[B_NODES * b:B_NODES * (b + 1)],
                            idxs_ap=gi_all[:, gi_off[c * NB + b]:
                                           gi_off[c * NB + b + 1]],
                            num_idxs=ndesc, num_idxs_reg=ndesc, elem_size=EL,
                            single_packet=False, queue_num=b,
                        )
                    g_ts.append(g_t)

                    mcb = int(M_cb[c, b])
                    s_t = sbuf_s.tile([128, mcb * W], _BF16, tag=f"s{b}")
                    if do_dve:
                        s3 = s_t[:].rearrange("p (m s) -> p m s", s=W)
                        mo = m_off[c * NB + b]
                        gs_b = gs_all[:, mo:mo + mcb].unsqueeze(
                            2).to_broadcast([128, mcb, W])
                        io_b = iota_t[:].unsqueeze(1).to_broadcast(
                            [128, mcb, W])
                        gv_b = gv_all[:, mo:mo + mcb].unsqueeze(
                            2).to_broadcast([128, mcb, W])
                        nc.vector.tensor_tensor(out=s3, in0=gs_b, in1=io_b,
                                                op=mybir.AluOpType.is_equal)
                        if not dve_half:
                            nc.vector.tensor_tensor(out=s3, in0=s3, in1=gv_b,
                                                    op=mybir.AluOpType.mult)
                    s_ts.append(s_t)

                if do_pe and do_dve and do_gather:
                    ps = psum.tile([128, TPC * D], _F32, space="PSUM",
                                   tag="ps")
                    for (b, t, w, m, st, sp) in plan[c]["entries"]:
                        a, j = w % GP, w // GP
                        nc.tensor.matmul(
                            out=ps[32 * a:32 * a + W, D * j:D * j + D],
                            lhsT=s_ts[b][:, W * m:W * (m + 1)],
                            rhs=g_ts[b][:, EL * t:EL * t + D],
                            start=st, stop=sp,
                            skip_group_check=True,
                        )
                    for w in plan[c]["missing"]:
                        a, j = w % GP, w // GP
                        nc.tensor.matmul(
                            out=ps[32 * a:32 * a + W, D * j:D * j + D],
                            lhsT=s_ts[0][:, :W],
                            rhs=g_ts[0][:, :D],
                            start=True, stop=True,
                            skip_group_check=True,
                        )

                    sc = scp.tile([96, TPC * D], _F32, tag="sc")
                    nc.scalar.copy(out=sc[:], in_=ps[:96])
                    out_eng = getattr(nc, out_engine)
                    out_eng.dma_start(
                        out=out_d[RPCH * c:RPCH * (c + 1), :].rearrange(
                            "(j a s) f -> (a s) j f", a=GP, s=W),
                        in_=sc[:].rearrange("p (j f) -> p j f", f=D),
                    )
    nc.compile()
    return nc


# ===========================================================================
# Entry point
# ===========================================================================
_CACHE = {}


def _get_program(meta, repeat=1):
    key = (repeat, meta["GIW_total"], meta["M_total"])
    if key not in _CACHE:
        _CACHE[key] = build_program(meta, repeat)
    return _CACHE[key]


def _run(adj_rows, adj_cols, adj_vals, x):
    x64 = pad_x(np.ascontiguousarray(np.asarray(x), dtype=np.float32))
    in_maps, meta = prep_inputs(adj_rows, adj_cols, adj_vals)
    for m in in_maps:
        m["x64"] = x64
    nc = _get_program(meta)
    res = run_bass_kernel_spmd(nc, in_maps, core_ids=list(range(N_CORES)))
    out = np.empty((N_NODES, D), np.float32)
    for k in range(N_CORES):
        out[k * R_PER_CORE:(k + 1) * R_PER_CORE] = \
            res.results[k]["out"][:R_PER_CORE]
    return out, res, (in_maps, meta)


def kernel(adj_rows, adj_cols, adj_vals, x):
    out, _, _ = _run(adj_rows, adj_cols, adj_vals, x)
    return out
